# revision 4
# baseline (speedup 1.0000x reference)
"""AttentionCropLayer Trainium2 kernel.

Per sample b: offsets (w,h) = floor(clip(locs[b]*224, 44, 180) - 44); output
out[b] = images[b, :, w:w+88, h:h+88] * mask, with mask the fixed 88x88
sigmoid-profile outer product.

In fp32 the sigmoid profile rounds to [0.5, 1-4.54e-5, 1, 1, ..., 1,
1-4.54e-5]: every interior mask value is exactly 1.0, so the mask multiply
reduces to scaling row 0 and column 0 of each crop by 0.5 (corner by 0.25).
The 1-4.54e-5 entries are approximated as 1.0 (rel err ~9e-5, tol 2e-2).

Strategy (pure data parallel, 8 cores x 16 samples):
  - host stages each core's slab channel-interleaved: flat[s, r, col, c] =
    images[s, c, r, col].  One crop row x all 16 channels is then 1408
    contiguous floats (5632B), so a sample's whole crop is 88 big
    zero-garbage descriptors instead of 16*88 small ones.
  - device: per sample one dynamic-offset DMA (partition = (s, r%8), free =
    (r//8, col, c)); per group of 4 samples two tiny DVE edge-scale ops
    (row 0 / col 0 of the crop *0.5, corner gets 0.25 via both), then one
    big contiguous store per group into out2[s, m, k, col, c]
  - host unshards with a pure transpose: out[s,c,8k+m,col] = out2[s,m,k,col,c]
"""

import sys

if "/opt/trn_rl_repo" not in sys.path:
    sys.path.insert(0, "/opt/trn_rl_repo")

import numpy as np

import concourse.bass as bass
import concourse.bacc as bacc
import concourse.mybir as mybir
from concourse import tile
from concourse.bass_utils import run_bass_kernel_spmd

TL = 44
CROP = 2 * TL          # 88
SCALE = 224.0
B, C, IN = 128, 16, 224
NCORES = 8
BPC = B // NCORES      # 16 samples per core
MAXOFF = IN - CROP     # 136
IMSZ = C * IN * IN     # elems per sample
FLATSZ = BPC * IMSZ
RPP = 8                # partition sub-index m = r % 8
KD = CROP // RPP       # 11 free blocks k = r // 8
CW = C * CROP          # 1408 elems: one crop row x all channels
RST = IN * C           # 3584: DRAM row stride in the interleaved layout
KST = RST * RPP        # 28672
PFREE = KD * CW        # 15488 elems per partition
GRP = 4                # samples per pipeline group
NGRP = BPC // GRP
MAXEOFF = (BPC - 1) * IMSZ + (MAXOFF * IN + MAXOFF) * C

_nc_cache = {}


def _build_nc():
    nc = bacc.Bacc(None)
    images = nc.declare_dram_parameter(
        "images", [1, FLATSZ], mybir.dt.float32, isOutput=False
    )
    offs = nc.declare_dram_parameter(
        "offs", [1, BPC], mybir.dt.int32, isOutput=False
    )
    out = nc.declare_dram_parameter(
        "out", [BPC, RPP, KD, CROP, C], mybir.dt.float32, isOutput=True
    )

    with tile.TileContext(nc) as tc:
        with tc.tile_pool(name="work", bufs=1) as wpool:
            # warm the dynamic-DMA path on both HWDGE rings with a dummy
            # register-offset read: the first dynamic DMA per ring pays a
            # ~10us one-time cold cost (bc-ucode load); absorb it while the
            # offset staging DMA is still in flight
            warm_regs = {
                "sync": nc.sync.alloc_register("o_reg_sp"),
                "scalar": nc.scalar.alloc_register("o_reg_act"),
            }
            for rk, weng in (("sync", nc.sync), ("scalar", nc.scalar)):
                wreg = warm_regs[rk]
                weng.reg_mov(wreg, 0)
                ov0 = weng.snap(wreg, donate=True, min_val=0, max_val=0)
                wsrc = bass.AP(
                    tensor=images[:].tensor,
                    offset=ov0,
                    ap=[[64, 16], [1, 64]],
                    dep_tracking_offset=0,
                )
                wt_ = wpool.tile([16, 64], mybir.dt.float32, tag=f"warm_{rk}")
                weng.dma_start(out=wt_[:], in_=wsrc)
            # offsets staged via SWDGE (no HWDGE cold cost in its path)
            offs_sb = wpool.tile([1, BPC], mybir.dt.int32)
            nc.gpsimd.dma_start(out=offs_sb[:], in_=offs[:])

            t = wpool.tile([BPC * RPP, PFREE], mybir.dt.float32, tag="t")
            t_ap = t[:]
            engs = {"sync": nc.sync, "scalar": nc.scalar}
            for g in range(NGRP):
                for j in range(GRP):
                    s = g * GRP + j
                    rk = "sync" if s % 2 == 0 else "scalar"
                    eng_, reg_ = engs[rk], warm_regs[rk]
                    eng_.reg_load(reg_, offs_sb[0:1, s : s + 1])
                    ov = eng_.snap(reg_, donate=True, min_val=0, max_val=MAXEOFF)
                    srcap = bass.AP(
                        tensor=images[:].tensor,
                        offset=ov,
                        ap=[[RST, RPP], [KST, KD], [1, CW]],
                        dep_tracking_offset=s * IMSZ,
                    )
                    eng_.dma_start(
                        out=t[s * RPP : (s + 1) * RPP, :], in_=srcap
                    )
                # edge fixups: crop row 0 lives in partitions m==0, free
                # [0:CW); crop col 0 is free [k*CW : k*CW+C) on every
                # partition.  Both *0.5; the corner gets both -> 0.25.
                for j in range(GRP):
                    s = g * GRP + j
                    p0 = s * RPP
                    nc.vector.tensor_scalar_mul(
                        t[p0 : p0 + 1, 0:CW], t[p0 : p0 + 1, 0:CW], 0.5
                    )
                gs = t[g * GRP * RPP : (g + 1) * GRP * RPP]
                gs_ap = gs[:]
                c0 = bass.AP(
                    tensor=gs_ap.tensor,
                    offset=gs_ap.offset,
                    ap=[gs_ap.ap[0], [CW, KD], [1, C]],
                )
                nc.vector.tensor_scalar_mul(c0, c0, 0.5)
                out_view = out[g * GRP : (g + 1) * GRP].rearrange(
                    "s m k col c -> (s m) (k col c)"
                )
                nc.gpsimd.dma_start(out=out_view, in_=gs[:])
    nc.finalize()
    return nc


def _get_nc():
    if "nc" not in _nc_cache:
        _nc_cache["nc"] = _build_nc()
    return _nc_cache["nc"]


def _host_offsets(locs):
    locs = np.asarray(locs, dtype=np.float32)
    t = np.clip(locs * np.float32(SCALE), np.float32(TL), np.float32(IN - TL))
    return np.floor(t - np.float32(TL)).astype(np.int32)  # [B, 2] (w, h)


def make_in_maps(images, locs):
    images = np.asarray(images, dtype=np.float32)
    off = _host_offsets(locs)  # [B, 2] (w, h)
    s_idx = np.arange(BPC, dtype=np.int64)
    in_maps = []
    for i in range(NCORES):
        sl = slice(i * BPC, (i + 1) * BPC)
        osh = off[sl].astype(np.int64)
        eoff = (s_idx * IMSZ + (osh[:, 0] * IN + osh[:, 1]) * C).astype(np.int32)
        # channel-interleaved slab: flat[s, r, col, c] = images[s, c, r, col]
        flat = np.ascontiguousarray(
            images[sl].transpose(0, 2, 3, 1)
        ).reshape(1, FLATSZ)
        in_maps.append(
            {
                "images": flat,
                "offs": np.ascontiguousarray(eoff.reshape(1, -1)),
            }
        )
    return in_maps


def run(images, locs, trace=False, **kwargs):
    nc = _get_nc()
    in_maps = make_in_maps(images, locs)
    res = run_bass_kernel_spmd(
        nc, in_maps, core_ids=list(range(NCORES)), trace=trace, **kwargs
    )
    outs = []
    for i in range(NCORES):
        o2 = np.asarray(res.results[i]["out"])  # [s, m, k, col, c]
        # out[s, c, 8k+m, col] = out2[s, m, k, col, c]
        o = o2.transpose(0, 4, 2, 1, 3).reshape(BPC, C, CROP, CROP)
        outs.append(o)
    full = np.concatenate(outs, axis=0).astype(np.float32)
    return full, res


def kernel(images, locs):
    full, _ = run(images, locs, trace=False)
    return full
